# revision 2
# baseline (speedup 1.0000x reference)
"""ConvKAN Trainium2 Bass kernel — v2 (packed elementwise + strip/chunk interleave).

Problem: nn_ConvKAN (B=8, C=64, H=W=64, OUT=64, 3x3 conv, KAN spline G=5 k=3).

Data-parallel over batch: core i handles image i. Host pads x to [C, 66, 66]
fp16 (zero borders) so the device never needs a memset and basis values at
pad cells are evaluated at x=0, matching the reference's zero-padded unfold.

Per core:
  Elementwise (strip-mined, 6 strips of 11 padded rows = 726 cells):
    u_t = |x - c_j|           (ACT Abs, per t: j = 2t + p//64)  -> packed [128, 4*726]
    A   = relu(alpha*(2-2.5u))   alpha = 6^(-1/3)               (ACT Relu, packed)
    Cb  = alpha*relu(a-1) = max(A, alpha) - alpha               (DVE ts, packed)
    qA  = A*A ; s1 = qA*A = a^3/6                               (DVE tt)
    qC  = Cb*Cb ; s2_4 = (qC*4)*Cb = (2/3) b^3                  (DVE tt + stt)
    B   = s1 - s2_4                                             (tt, strided out)
    silu channel on ACT.
  Matmul: 6 strip-aligned chunk pairs (rows 5+4, then 5x(6+5)), 45 K-tile
  steps per pair (9 taps x (4 basis K=128 + 1 silu K=64)), col-group packed
  (even chunk on PE cols 0-63, odd on 64-127). PSUM -> fp16 SBUF -> DMA out.
  Output returned fp16, host casts to fp32.
"""
import os
import sys

sys.path.insert(0, "/opt/trn_rl_repo")

import numpy as np

import concourse.bass as bass
import concourse.bacc as bacc
import concourse.tile as tile
from concourse import mybir
from concourse.bass_utils import run_bass_kernel_spmd

# ---- problem constants (hardcoded per contest rules) ----
B, C, H, W = 8, 64, 64, 64
OUT_CH = 64
NJ = 8
HP, WP = H + 2, W + 2
S = HP * WP                 # 4356
RW = S + 4                  # rhs tile width: data at col 2 (4B-aligned strips), garbage at 0,1 and tail
ALPHA = 6.0 ** (-1.0 / 3.0)
TAU = 2.5                   # 1/grid_h
F32 = mybir.dt.float32
F16 = mybir.dt.float16

# ragged strips: small first/last strip to shorten pipeline lead-in/tail.
# Each entry is (padded_row_start, padded_row_count).
STRIPS = [(0, 6), (6, 11), (17, 11), (28, 11), (39, 11), (50, 11), (61, 5)]
N_STRIPS = len(STRIPS)
PKMAX = 4 * 11 * WP         # max packed elementwise width
# chunk pairs (start_row, even_rows, odd_rows); matmul windows split at
# strip boundaries, so pairs need not align to strips. Pair p is emitted
# after the strip containing its halo row (start+re+ro+1).
PAIRS = [(0, 5, 4)] + [(9 + 11 * k, 6, 5) for k in range(5)]
assert PAIRS[-1][0] + PAIRS[-1][1] + PAIRS[-1][2] == H

C4 = 4.0 ** (1.0 / 3.0)     # Cd = C4 * Cb so Cd^3 = 4 alpha^3 b^3 = (2/3) b^3

_CACHE = {}


def _fold_weights(base_weight, spline_weight, spline_scaler):
    """Host-side weight prep into lhsT layout [128, 45*64] fp16.

    K-tile t in 0..3 of each tap holds basis channels j=2t (partitions 0:64,
    c-major) and j=2t+1 (partitions 64:128); k-tile 4 holds the silu channel
    (partitions 0:64). Block index = (kh*3+kw)*5 + t.
    """
    sw = (spline_weight.astype(np.float64) * spline_scaler.astype(np.float64)[:, :, None])
    sw4 = sw.reshape(OUT_CH, C, 9, NJ)
    bw4 = base_weight.astype(np.float64).reshape(OUT_CH, C, 9)
    Wk = np.zeros((128, 45, 64), np.float64)
    for s9 in range(9):
        for t in range(4):
            for half in range(2):
                j = 2 * t + half
                Wk[half * 64:(half + 1) * 64, s9 * 5 + t, :] = sw4[:, :, s9, j].T
        Wk[0:64, s9 * 5 + 4, :] = bw4[:, :, s9].T
    return Wk.reshape(128, 45 * 64).astype(np.float16)


def _ctab():
    # col t (t=0..3): -c_j for j = 2t + p//64 (bias for ACT Abs: u=|x-c|)
    # col 4: 2*alpha (bias for the Relu producing A = alpha*relu(2-2.5u))
    ct = np.zeros((128, 5), np.float32)
    for t in range(4):
        for p in range(128):
            j = 2 * t + p // 64
            ct[p, t] = -(j - 3.5) / TAU
    ct[:, 4] = 2.0 * ALPHA
    return ct


def _build_nc():
    nc = bacc.Bacc()
    x_ext = nc.dram_tensor("x_img", [C, S], F16, kind="ExternalInput")
    wk_ext = nc.dram_tensor("wk", [128, 45 * 64], F16, kind="ExternalInput")
    ct_ext = nc.dram_tensor("ct", [128, 5], F32, kind="ExternalInput")
    out_ext = nc.dram_tensor("out", [OUT_CH, H, W], F16, kind="ExternalOutput")

    # per-strip block width (data at col 2, garbage cols 0,1 and tail pad)
    BW = [r * WP + 4 for _, r in STRIPS]

    def strip_of_row(r):
        for k, (r0, n) in enumerate(STRIPS):
            if r0 <= r < r0 + n:
                return k
        raise ValueError(r)

    with tile.TileContext(nc) as tc:
        with (
            tc.tile_pool(name="const", bufs=1) as const_pool,
            tc.tile_pool(name="tin", bufs=3) as tin_pool,
            tc.tile_pool(name="qa", bufs=3) as qa_pool,
            tc.tile_pool(name="temps", bufs=2) as temp_pool,
            tc.tile_pool(name="outs", bufs=4) as out_pool,
            tc.tile_pool(name="psum", bufs=3, space="PSUM") as psum_pool,
        ):
            ctab = const_pool.tile([128, 5], F32, tag="ctab")
            nc.sync.dma_start(out=ctab[:, :], in_=ct_ext[:, :])

            # per-strip tiles: writers never touch tiles already read by PE
            xxs = [const_pool.tile([128, r * WP], F16, tag=f"xx{k}",
                                   name=f"xx{k}")
                   for k, (_, r) in enumerate(STRIPS)]
            ralls = [const_pool.tile([128, 4 * BW[k]], F16, tag=f"ra{k}",
                                     name=f"ra{k}")
                     for k in range(N_STRIPS)]
            rsils = [const_pool.tile([64, BW[k]], F16, tag=f"rs{k}",
                                     name=f"rs{k}")
                     for k in range(N_STRIPS)]

            # strip 0's x first so compute starts ASAP, then weights
            r00, n00 = STRIPS[0]
            nc.sync.dma_start(out=xxs[0][0:64, :], in_=x_ext[:, r00 * WP:(r00 + n00) * WP])
            nc.sync.dma_start(out=xxs[0][64:128, :], in_=x_ext[:, r00 * WP:(r00 + n00) * WP])
            wsb = const_pool.tile([128, 45 * 64], F16, tag="wsb")
            nc.sync.dma_start(out=wsb[:, :], in_=wk_ext[:, :])

            def emit_strip(sp):
                row0, rows = STRIPS[sp]
                cells = rows * WP
                pk = 4 * cells
                xx = xxs[sp]
                if sp > 0:
                    nc.sync.dma_start(out=xx[0:64, :], in_=x_ext[:, row0 * WP:row0 * WP + cells])
                    nc.sync.dma_start(out=xx[64:128, :], in_=x_ext[:, row0 * WP:row0 * WP + cells])

                u = tin_pool.tile([128, PKMAX], F16, tag="u")
                for t in range(4):
                    nc.scalar.activation(
                        u[:, t * cells:(t + 1) * cells], xx[:, :],
                        mybir.ActivationFunctionType.Abs,
                        bias=ctab[:, t:t + 1], scale=1.0)
                A = tin_pool.tile([128, PKMAX], F16, tag="A")
                nc.scalar.activation(A[:, 0:pk], u[:, 0:pk],
                                     mybir.ActivationFunctionType.Relu,
                                     bias=ctab[:, 4:5], scale=-TAU * ALPHA)
                # qA = (alpha*(2-2.5u))^2 straight from u; the u>0.8 ghost is
                # killed by the relu'd A factor in s1.
                qA = qa_pool.tile([128, PKMAX], F16, tag="qA")
                nc.scalar.activation(qA[:, 0:pk], u[:, 0:pk],
                                     mybir.ActivationFunctionType.Square,
                                     bias=ctab[:, 4:5], scale=-TAU * ALPHA)
                Cm = temp_pool.tile([128, PKMAX], F16, tag="Cm")
                nc.vector.tensor_scalar(
                    Cm[:, 0:pk], A[:, 0:pk], C4, C4 * ALPHA,
                    mybir.AluOpType.mult, mybir.AluOpType.max)
                Cd = temp_pool.tile([128, PKMAX], F16, tag="Cd")
                nc.vector.tensor_scalar(
                    Cd[:, 0:pk], Cm[:, 0:pk], -C4 * ALPHA, 1.0,
                    mybir.AluOpType.add, mybir.AluOpType.mult)
                s1t = temp_pool.tile([128, PKMAX], F16, tag="s1t")
                nc.vector.tensor_mul(s1t[:, 0:pk], qA[:, 0:pk], A[:, 0:pk])
                qC = temp_pool.tile([128, PKMAX], F16, tag="qC")
                nc.vector.tensor_mul(qC[:, 0:pk], Cd[:, 0:pk], Cd[:, 0:pk])
                s24 = temp_pool.tile([128, PKMAX], F16, tag="s24")
                nc.vector.tensor_mul(s24[:, 0:pk], qC[:, 0:pk], Cd[:, 0:pk])
                rv = ralls[sp][:, :].rearrange("p (t r) -> p t r", r=BW[sp])
                s1v = s1t[:, 0:pk].rearrange("p (t r) -> p t r", r=cells)
                s24v = s24[:, 0:pk].rearrange("p (t r) -> p t r", r=cells)
                nc.vector.tensor_tensor(
                    rv[:, :, 2:2 + cells], s1v[:, :, :], s24v[:, :, :],
                    mybir.AluOpType.subtract)
                nc.scalar.activation(rsils[sp][:, 2:2 + cells], xx[0:64, :],
                                     mybir.ActivationFunctionType.Silu)

            pair_psum = {}

            def emit_chunk_mms(ps, prange, y0, nrows, s9, t, first, last):
                kh, kw = s9 // 3, s9 % 3
                kdim = 128 if t < 4 else 64
                blk = (s9 * 5 + t) * 64
                # rhs rows y0+kh .. y0+kh+nrows-1, split at strip boundaries
                r = y0 + kh
                rem = nrows
                seg_first = True
                while rem > 0:
                    k = strip_of_row(r)
                    r0k, nk = STRIPS[k]
                    seg = min(rem, r0k + nk - r)
                    n = seg * WP
                    c0 = (r - (y0 + kh)) * WP
                    if t < 4:
                        off = t * BW[k] + 1 + (r - r0k) * WP + kw
                        rhs = ralls[k][0:kdim, off:off + n]
                    else:
                        off = 1 + (r - r0k) * WP + kw
                        rhs = rsils[k][0:kdim, off:off + n]
                    nc.tensor.matmul(
                        ps[prange[0]:prange[1], c0:c0 + n],
                        wsb[0:kdim, blk:blk + 64], rhs,
                        start=(first and seg_first), stop=(last and rem == seg),
                        tile_position=(0, prange[0]))
                    seg_first = False
                    r += seg
                    rem -= seg

            def emit_pair_mms(p):
                y0, re_, ro_ = PAIRS[p]
                y0o = y0 + re_
                ps = psum_pool.tile([128, 6 * WP], F32, tag="ps")
                pair_psum[p] = ps
                for s9 in range(9):
                    for t in range(5):
                        first = (s9 == 0 and t == 0)
                        last = (s9 == 8 and t == 4)
                        emit_chunk_mms(ps, (0, 64), y0, re_, s9, t, first, last)
                        emit_chunk_mms(ps, (64, 128), y0o, ro_, s9, t, first, last)

            def emit_pair_drain(p):
                y0, re_, ro_ = PAIRS[p]
                y0o = y0 + re_
                ne, no = re_ * WP, ro_ * WP
                ps = pair_psum.pop(p)
                oe = out_pool.tile([64, 6 * WP], F16, tag="oe")
                nc.scalar.copy(oe[:, 0:ne], ps[0:64, 0:ne])
                oev = oe[:, 0:ne].rearrange("p (r w) -> p r w", w=WP)
                nc.sync.dma_start(out=out_ext[:, y0:y0 + re_, :],
                                  in_=oev[:, :, 1:65])
                oo = out_pool.tile([64, 6 * WP], F16, tag="oo")
                nc.vector.tensor_copy(oo[:, 0:no], ps[64:128, 0:no])
                oov = oo[:, 0:no].rearrange("p (r w) -> p r w", w=WP)
                nc.sync.dma_start(out=out_ext[:, y0o:y0o + ro_, :],
                                  in_=oov[:, :, 1:65])

            # pair p emitted after the strip holding its last halo row
            pair_after = {}
            for p, (y0, re_, ro_) in enumerate(PAIRS):
                pair_after.setdefault(strip_of_row(y0 + re_ + ro_ + 1), []).append(p)
            drain_q = []
            for sp in range(N_STRIPS):
                emit_strip(sp)
                while drain_q:
                    emit_pair_drain(drain_q.pop(0))
                for p in pair_after.get(sp, []):
                    emit_pair_mms(p)
                    drain_q.append(p)
            while drain_q:
                emit_pair_drain(drain_q.pop(0))
    nc.finalize()
    return nc


def _fast_runner(nc):
    """Cached jitted SPMD executor (same lowering as bass2jax.run_bass_via_pjrt
    multi-core path) with device-side zero output buffers and no per-call
    retracing."""
    import jax
    import jax.numpy as jnp
    from jax.experimental.shard_map import shard_map
    from jax.sharding import Mesh, NamedSharding, PartitionSpec
    from concourse import bass2jax

    bass2jax.install_neuronx_cc_hook()

    partition_name = (nc.partition_id_tensor.name
                      if nc.partition_id_tensor else None)
    in_names, out_names, out_avals = [], [], []
    for alloc in nc.m.functions[0].allocations:
        if not isinstance(alloc, mybir.MemoryLocationSet):
            continue
        name = alloc.memorylocations[0].name
        if alloc.kind == "ExternalInput":
            if name != partition_name:
                in_names.append(name)
        elif alloc.kind == "ExternalOutput":
            out_names.append(name)
            out_avals.append(jax.core.ShapedArray(
                tuple(alloc.tensor_shape), mybir.dt.np(alloc.dtype)))
    n_params = len(in_names)
    all_names = tuple(in_names) + tuple(out_names)
    if partition_name is not None:
        all_names = all_names + (partition_name,)
    n_outs = len(out_names)

    def _body(*args):
        operands = list(args)
        if partition_name is not None:
            operands.append(bass2jax.partition_id_tensor())
        outs = bass2jax._bass_exec_p.bind(
            *operands,
            out_avals=tuple(out_avals),
            in_names=all_names,
            out_names=tuple(out_names),
            lowering_input_output_aliases=(),
            sim_require_finite=True,
            sim_require_nnan=True,
            nc=nc,
        )
        return tuple(outs)

    devices = jax.devices()[:B]
    mesh = Mesh(np.asarray(devices), ("core",))
    in_specs = (PartitionSpec("core"),) * (n_params + n_outs)
    out_specs = (PartitionSpec("core"),) * n_outs
    donate = tuple(range(n_params, n_params + n_outs))
    sharded = jax.jit(
        shard_map(_body, mesh=mesh, in_specs=in_specs, out_specs=out_specs,
                  check_rep=False),
        donate_argnums=donate, keep_unused=True)
    zero_shardings = [NamedSharding(mesh, PartitionSpec("core"))] * n_outs

    def make_zeros():
        return [jnp.zeros((B * a.shape[0], *a.shape[1:]), a.dtype)
                for a in out_avals]

    zeros_fn = jax.jit(make_zeros, out_shardings=zero_shardings)
    return sharded, zeros_fn, list(in_names), list(out_names), out_avals


def kernel(x, base_weight, spline_weight, spline_scaler):
    x = np.asarray(x, dtype=np.float32)
    wk = _fold_weights(np.asarray(base_weight), np.asarray(spline_weight),
                       np.asarray(spline_scaler))
    ct = _ctab()

    x16 = np.zeros((B, C, HP, WP), np.float16)
    x16[:, :, 1:65, 1:65] = x
    x16 = np.ascontiguousarray(x16.reshape(B, C, S))

    if "nc" not in _CACHE:
        _CACHE["nc"] = _build_nc()
    nc = _CACHE["nc"]

    in_maps = [{"x_img": x16[i], "wk": wk, "ct": ct} for i in range(B)]

    if os.environ.get("BASS_TRACE", "") == "1":
        res = run_bass_kernel_spmd(nc, in_maps, list(range(B)))
        _CACHE["last_res"] = res
        out = np.stack([res.results[i]["out"] for i in range(B)], axis=0)
        return out.astype(np.float32)

    if not _CACHE.get("runner_broken"):
        try:
            if "runner" not in _CACHE:
                _CACHE["runner"] = _fast_runner(nc)
            sharded, zeros_fn, in_names, out_names, out_avals = _CACHE["runner"]
            concat_in = [np.ascontiguousarray(
                np.concatenate([m[name] for m in in_maps], axis=0))
                for name in in_names]
            outs = sharded(*concat_in, *zeros_fn())
            oi = out_names.index("out")
            out = np.asarray(outs[oi]).reshape(B, OUT_CH, H, W)
            return out.astype(np.float32)
        except Exception:
            _CACHE["runner_broken"] = True

    res = run_bass_kernel_spmd(nc, in_maps, list(range(B)))
    out = np.stack([res.results[i]["out"] for i in range(B)], axis=0)
    return out.astype(np.float32)


if __name__ == "__main__":
    rng = np.random.default_rng(0)
    ins = {
        "x": rng.standard_normal((B, C, H, W), dtype=np.float32),
        "base_weight": (rng.standard_normal((OUT_CH, 576)) * 0.05).astype(np.float32),
        "spline_weight": (rng.standard_normal((OUT_CH, 576, NJ)) * 0.05).astype(np.float32),
        "spline_scaler": (rng.standard_normal((OUT_CH, 576)) * 0.05).astype(np.float32),
    }
    o = kernel(**ins)
    print("kernel out:", o.shape, o.dtype, float(np.abs(o).max()))
